# revision 2
# baseline (speedup 1.0000x reference)
"""Trainium2 Bass kernel for a dense 16-head attention block (v2).

Per batch b (data-parallel over 8 NeuronCores, no collectives):
    qkv = x @ w_qkv; q,k,v split into 16 heads of 64
    attn = softmax((q*scale) @ k.T); out = (attn @ v) @ w_proj + b_proj

Layout: everything transposed (feature dim on partitions, tokens on the free
axis) so all matmuls contract over the partition dim.

Schedule / precision design (driven by the TimelineSim cost model, where a
matmul costs out_free_size * pe_cycle * cycles_per_row, fp8e4 DoubleRow
costing 0.5 cycles/row with 2x128 contraction per pass):

  - QKV generation runs as 3-term fp8 DoubleRow: host splits x^T and
    16*w_qkv into fp8e4m3 value+residual pairs (x8,xr / w8,wr) and the
    kernel computes x8@w8 + x8@wr + xr@w8 (the dropped xr@wr term is
    O(eps^2)).  The 16x weight scale keeps w_qkv out of the fp8 subnormal
    range; it is compensated for free: the softmax "ones" columns are 16.0
    (denominator absorbs the V scale) and the exp scale absorbs 1/256 for
    the Q.K logits.
  - S^T / attn@V / proj matmuls are fp16 (rel err ~3e-3 overall).
  - Attention runs in (head-pair, q-half) streams: S^T -> exp (ScalarE,
    [128,512] tiles) -> AV accumulate; the softmax denominator rides for
    free in PSUM rows 64..127 via the ones columns.  Normalization is
    DVE reciprocal + Pool (gpsimd) multiply.
  - The ACT-bound inner loop leaves ~370ns/iter of PE idle; a deadline-
    ordered fill queue of deferred QK/V generation units (and, for the
    last pair, proj tiles t=0..3 which only need q-half-0 norms) keeps the
    PE busy.  A dummy-matmul warmup chain covers the DMA prologue and the
    PE p-state ramp.

PSUM budget (8 banks): pss 2 x [128,512] (S tiles), psA 2 x [128,512]
(QK/V fill + proj), pso 4 x [128,512] (AV accumulators, two streams).
"""

import numpy as np
import ml_dtypes

P = 128
N = 1024          # tokens per core (= seq len)
D = 1024          # model dim
H = 16            # heads
DH = D // H       # 64
NCORES = 8
KD = D // P       # 8 contraction chunks
TT = N // P       # 8 token chunks
NH = 512          # matmul free-dim chunk
WS = 16.0         # host-side w_qkv scale (fp8 subnormal avoidance)
SCALE = (DH ** -0.5) / (WS * WS)   # exp scale absorbs the 16x q and k scales

NWARM = 24        # warmup matmuls covering DMA prologue + p-state ramp
FILL_PER_ITER = 480.0   # ns of fill-queue work drained per inner iteration

_BF16 = ml_dtypes.bfloat16
_F8 = ml_dtypes.float8_e4m3
_F16 = np.float16

_runner_cache = {}
DEBUG_DUMP = False


def _build_nc(use_mask: bool, use_bias: bool):
    import concourse.bass as bass
    import concourse.mybir as mybir
    import concourse.tile as tile
    from concourse import bacc

    f16 = mybir.dt.float16
    f8 = mybir.dt.float8e4
    f32 = mybir.dt.float32
    Exp = mybir.ActivationFunctionType.Exp
    DR = mybir.MatmulPerfMode.DoubleRow

    nc = bacc.Bacc("TRN2", target_bir_lowering=False, debug=False)

    # DRAM inputs.  x8/xr: x^T chunk-major [128, kc*1024+tok].
    # w8/wr: 16*w_qkv as [128, b*1024 + kc*128 + c] with col-blocks of 128 in
    # PAIR-MAJOR order b = 3*p + j for original block m = p + 8*j, so each
    # head pair's Q/K/V blocks are one contiguous 3KB DMA.
    x8d = nc.dram_tensor("x8", [P, KD * N], f8, kind="ExternalInput")
    xrd = nc.dram_tensor("xr", [P, KD * N], f8, kind="ExternalInput")
    w8d = nc.dram_tensor("w8", [P, 24 * KD * P], f8, kind="ExternalInput")
    wrd = nc.dram_tensor("wr", [P, 24 * KD * P], f8, kind="ExternalInput")
    wp8d = nc.dram_tensor("wp8", [P, KD * D], f8, kind="ExternalInput")
    wprd = nc.dram_tensor("wpr", [P, KD * D], f8, kind="ExternalInput")
    wpfd = nc.dram_tensor("wpf", [P, 2 * D], f16, kind="ExternalInput")
    if use_mask:
        # masked-softmax: exp(S)*m + (1-m) == exp(where(m, S, 0)); a fully
        # masked query row softmaxes to uniform, matching the reference.
        mask_bc = nc.dram_tensor("mask_bc", [P, N], f16, kind="ExternalInput")
        imask_bc = nc.dram_tensor("imask_bc", [P, N], f16, kind="ExternalInput")
    if use_bias:
        b_bc = nc.dram_tensor("b_bc", [P, D], f32, kind="ExternalInput")
    out = nc.dram_tensor("out", [N, D], f16, kind="ExternalOutput")
    if DEBUG_DUMP:
        qk_dbg = nc.dram_tensor("qk_dbg", [P, N], f16, kind="ExternalOutput")
        kk_dbg = nc.dram_tensor("kk_dbg", [P, N], f16, kind="ExternalOutput")
        v_dbg = nc.dram_tensor("v_dbg", [P, 2 * P], f16, kind="ExternalOutput")
        aot_dbg = nc.dram_tensor("aot_dbg", [P, N], f8, kind="ExternalOutput")

    with tile.TileContext(nc) as tc:
        with (
            tc.tile_pool(name="persist", bufs=1) as pp,
            tc.tile_pool(name="pt", bufs=6) as ptp,
            tc.tile_pool(name="nrm", bufs=4) as nrm,
            tc.tile_pool(name="ob", bufs=4) as obp,
            tc.tile_pool(name="psA", bufs=2, space="PSUM") as psA,
            tc.tile_pool(name="pss", bufs=2, space="PSUM") as pss,
            tc.tile_pool(name="pso", bufs=4, space="PSUM") as pso,
        ):
            X8 = pp.tile([P, KD * N], f8, name="x8t")
            XR = pp.tile([P, KD * N], f8, name="xrt")
            W8 = pp.tile([P, 24 * KD * P], f8, name="w8t")
            WR = pp.tile([P, 24 * KD * P], f8, name="wrt")
            WP8 = pp.tile([P, KD * D], f8, name="wp8t")
            WPR = pp.tile([P, KD * D], f8, name="wprt")
            WPF = pp.tile([P, 2 * D], f16, name="wpft")
            QK = [pp.tile([P, N], f16, name=f"qk{m}") for m in range(16)]
            # V per (token tile, head pair) so just-in-time generation writes
            # don't alias the AV reads of other pairs' blocks
            V = [[pp.tile([P, 2 * P], f16, name=f"v{t}_{p}") for p in range(8)]
                 for t in range(TT)]
            # 16x-scaled attention output: chunk-pairs 0..2 (pairs 0..5) as
            # fp8 value+residual for the DR proj passes; pairs 6,7 stay fp16
            # so the last norms skip the quantize chain entirely
            A8 = [pp.tile([P, 2 * N], f8, name=f"a8t{c}") for c in range(3)]
            AR = [pp.tile([P, 2 * N], f8, name=f"art{c}") for c in range(3)]
            AF = [pp.tile([P, N], f16, name=f"aft{k}") for k in (6, 7)]
            wa = pp.tile([P, P], f16, name="warm_a")
            nc.vector.memset(wa[:], 0.0)

            # DMA order = first-use order, with few large transfers (each
            # dma_start costs ~565ns of serial SP issue time): x8 halves +
            # pair-0 weights first, then xr, then per-pair weight blocks.
            def dma_wpair(tile_, dram, p, eng=None):
                sl = slice(3 * p * KD * P, 3 * (p + 1) * KD * P)
                (eng or nc.sync).dma_start(out=tile_[:, sl], in_=dram[:, sl])

            half = KD * N // 2
            nc.sync.dma_start(out=X8[:, :half], in_=x8d[:, :half])
            dma_wpair(W8, w8d, 0)
            dma_wpair(WR, wrd, 0)
            nc.sync.dma_start(out=X8[:, half:], in_=x8d[:, half:])
            nc.sync.dma_start(out=XR[:, :half], in_=xrd[:, :half])
            nc.sync.dma_start(out=XR[:, half:], in_=xrd[:, half:])
            for p in range(1, 8):
                dma_wpair(W8, w8d, p)
                dma_wpair(WR, wrd, p)
            nc.sync.dma_start(out=WP8[:], in_=wp8d[:])
            nc.sync.dma_start(out=WPR[:], in_=wprd[:])
            nc.sync.dma_start(out=WPF[:], in_=wpfd[:])
            if use_mask:
                mbc = pp.tile([P, N], f16, name="mbc")
                nc.sync.dma_start(out=mbc[:], in_=mask_bc[:])
                imbc = pp.tile([P, N], f16, name="imbc")
                nc.sync.dma_start(out=imbc[:], in_=imask_bc[:])
            if use_bias:
                bbc = pp.tile([P, D], f32, name="bbc")
                nc.sync.dma_start(out=bbc[:], in_=b_bc[:])

            # ones columns of the V tiles, on the otherwise-idle Pool
            # (1.0, not WS: leaves AOT scaled by 16x = WS for the fp8 split)
            for t in range(TT):
                for p in range(8):
                    ones_view = V[t][p].rearrange("p (h c) -> p h c", c=P)[:, :, DH:]
                    nc.gpsimd.memset(ones_view, 1.0)

            # p-state warmup: dummy matmul chain on the PE during DMA wait
            wps = pso.tile([P, P], f32, tag="pso", name="warm_ps")
            for _ in range(NWARM):
                nc.tensor.matmul(wps[:], lhsT=wa[:], rhs=wa[:],
                                 start=True, stop=True)

            X8v = X8.rearrange("p (kc t) -> p kc t", t=N)
            XRv = XR.rearrange("p (kc t) -> p kc t", t=N)
            W8v = W8.rearrange("p (m kc c) -> p m kc c", kc=KD, c=P)
            WRv = WR.rearrange("p (m kc c) -> p m kc c", kc=KD, c=P)
            WP8v = WP8.rearrange("p (k d) -> p k d", d=D)
            WPRv = WPR.rearrange("p (k d) -> p k d", d=D)
            A8v = [a.rearrange("p (i t) -> p i t", t=N) for a in A8]
            ARv = [a.rearrange("p (i t) -> p i t", t=N) for a in AR]
            WPFv = WPF.rearrange("p (k d) -> p k d", d=D)

            TERMS = ((X8v, W8v), (X8v, WRv), (XRv, W8v))

            def emit_qk_unit(m, half):
                """QK tile m (of 16), token half: 12 DoubleRow matmuls."""
                b = 3 * (m % 8) + m // 8   # pair-major weight block index
                sl = slice(half * NH, (half + 1) * NH)
                ps = psA.tile([P, NH], f32, tag="psA", name=f"psqk{m}_{half}")
                last = len(TERMS) - 1
                for ti, (xv, wv) in enumerate(TERMS):
                    for c in range(KD // 2):
                        nc.tensor.matmul(
                            ps[:],
                            lhsT=wv[:, b, 2 * c:2 * c + 2, :],
                            rhs=xv[:, 2 * c:2 * c + 2, sl],
                            start=(ti == 0 and c == 0),
                            stop=(ti == last and c == KD // 2 - 1),
                            perf_mode=DR,
                        )
                nc.vector.tensor_copy(QK[m][:, sl], ps[:])

            def emit_v_unit(t, p):
                """V cols for head pair p, token tile t: 12 tiny DR matmuls."""
                ps = psA.tile([P, P], f32, tag="psA", name=f"psv{t}_{p}")
                last = len(TERMS) - 1
                for ti, (xv, wv) in enumerate(TERMS):
                    for c in range(KD // 2):
                        nc.tensor.matmul(
                            ps[:],
                            lhsT=xv[:, 2 * c:2 * c + 2, t * P:(t + 1) * P],
                            rhs=wv[:, 3 * p + 2, 2 * c:2 * c + 2, :],
                            start=(ti == 0 and c == 0),
                            stop=(ti == last and c == KD // 2 - 1),
                            perf_mode=DR,
                        )
                dest = V[t][p].rearrange("p (h c) -> p h c", c=P)[:, :, :DH]
                nc.vector.tensor_copy(dest, ps.rearrange("p (i c) -> p i c", c=DH))

            PTERMS = ((A8v, WP8v), (A8v, WPRv), (ARv, WP8v))

            def emit_proj(t, j):
                """proj output tile: tokens t*128.., dims j*512..: chunks 0..5
                as 9 DR mm on the 16x-scaled fp8 value+residual pairs, chunks
                6,7 as fp16 mm on the 16x-scaled AOT, then a 1/256 scale."""
                ps = psA.tile([P, NH], f32, tag="psA", name=f"ps3_{t}_{j}")
                for ti, (av, wv) in enumerate(PTERMS):
                    for c in range(3):
                        nc.tensor.matmul(
                            ps[:],
                            lhsT=av[c][:, :, t * P:(t + 1) * P],
                            rhs=wv[:, 2 * c:2 * c + 2, j * NH:(j + 1) * NH],
                            start=(ti == 0 and c == 0),
                            stop=False,
                            perf_mode=DR,
                        )
                for k in (6, 7):
                    nc.tensor.matmul(
                        ps[:],
                        lhsT=AF[k - 6][:, t * P:(t + 1) * P],
                        rhs=WPFv[:, k - 6, j * NH:(j + 1) * NH],
                        start=False, stop=(k == 7),
                    )
                ob = obp.tile([P, NH], f16, tag="ob", name=f"ob{t}_{j}")
                nc.vector.tensor_scalar_mul(ob[:], ps[:], 1.0 / (WS * WS))
                if use_bias:
                    nc.vector.tensor_add(ob[:], ob[:], bbc[:, j * NH:(j + 1) * NH])
                nc.sync.dma_start(out=out[t * P:(t + 1) * P, j * NH:(j + 1) * NH],
                                  in_=ob[:])

            # ---- fill queue: deferred work units in deadline order ----
            QK_COST, V_COST, PROJ_COST = 1280.0, 320.0, 1350.0
            fill_q = []
            for t in range(1, TT):
                fill_q.append((V_COST, (emit_v_unit, t, 0)))
            fill_q.append((QK_COST, (emit_qk_unit, 0, 1)))
            for p in range(1, 8):
                fill_q.append((QK_COST, (emit_qk_unit, p, 0)))
                fill_q.append((QK_COST, (emit_qk_unit, 8 + p, 0)))
                fill_q.append((QK_COST, (emit_qk_unit, 8 + p, 1)))
                for t in range(TT):
                    fill_q.append((V_COST, (emit_v_unit, t, p)))
                fill_q.append((QK_COST, (emit_qk_unit, p, 1)))
            fill_q.reverse()   # pop from the end
            # proj tiles t<4 only need q-half-0 norms: they fill the last
            # pair's otherwise-dry streams (gated to after norm(7, h0))
            proj_q = [(PROJ_COST, (emit_proj, t, j))
                      for t in range(3, -1, -1) for j in (1, 0)]

            state = {"credit": 0.0}

            def drain(ns, q=fill_q):
                state["credit"] += ns
                while q and q[-1][0] <= state["credit"]:
                    cost, (fn, *args) = q.pop()
                    fn(*args)
                    state["credit"] -= cost

            def stream(p, half, fillq, fill_ns=FILL_PER_ITER):
                """Attention for head pair p, query half: S->exp->AV over kt
                (1-iter software pipeline), then normalize into AOT[p]."""
                sl = slice(half * NH, (half + 1) * NH)
                qtile, ktile = QK[p], QK[8 + p]
                psos = [pso.tile([P, NH], f32, tag="pso", name=f"pso{p}_{half}_{i}")
                        for i in range(2)]
                pts = {}
                for kt in range(TT + 1):
                    if kt < TT:
                        for i in range(2):
                            pr = i * DH
                            ps_s = pss.tile([P, NH], f32, tag="pss",
                                            name=f"pss{p}_{half}_{kt}_{i}")
                            nc.tensor.matmul(
                                ps_s[:],
                                lhsT=ktile[pr:pr + DH, kt * P:(kt + 1) * P],
                                rhs=qtile[pr:pr + DH, sl],
                                start=True, stop=True,
                            )
                            pt = ptp.tile([P, NH], f16, tag="pt",
                                          name=f"pt{p}_{half}_{kt}_{i}")
                            nc.scalar.activation(pt[:], ps_s[:], Exp, scale=SCALE)
                            if use_mask:
                                nc.vector.tensor_mul(pt[:], pt[:], mbc[:, sl])
                                nc.vector.tensor_add(pt[:], pt[:], imbc[:, sl])
                            pts[kt, i] = pt
                    if kt > 0:
                        for i in range(2):
                            nc.tensor.matmul(
                                psos[i][:],
                                lhsT=V[kt - 1][p][:, i * P:(i + 1) * P],
                                rhs=pts.pop((kt - 1, i))[:],
                                start=(kt - 1 == 0), stop=(kt - 1 == TT - 1),
                            )
                    drain(fill_ns, fillq)
                if p < 6:
                    # full-height staging tile: head i's rows live at base
                    # partition i*64 so the SB+SB quantize ops are aligned
                    t32 = nrm.tile([P, NH], f32, tag="t32",
                                   name=f"t32_{p}_{half}")
                for i in range(2):
                    pr = i * DH
                    rec = nrm.tile([DH, NH], f32, tag="rec",
                                   name=f"rec{p}_{half}_{i}")
                    nc.vector.reciprocal(rec[:], psos[i][DH:2 * DH, :])
                    if p >= 6:
                        nc.vector.tensor_mul(AF[p - 6][pr:pr + DH, sl],
                                             psos[i][:DH, :], rec[:])
                    else:
                        nc.vector.tensor_mul(t32[pr:pr + DH, :],
                                             psos[i][:DH, :], rec[:])
                        a8sl = A8v[p // 2][pr:pr + DH, p % 2, sl]
                        nc.gpsimd.tensor_copy(a8sl, t32[pr:pr + DH, :])
                        nc.gpsimd.tensor_sub(ARv[p // 2][pr:pr + DH, p % 2, sl],
                                             t32[pr:pr + DH, :], a8sl)

            # prologue compute: pair 0's Q,K (half 0 + K half 1) and V(t=0).
            # Four psum groups (psA x2 + idle pss x2) stay open and the fp8
            # term-passes are interleaved chunk-first, so matmuls start as
            # soon as each DMA lands instead of waiting for xr.
            pro = {}

            def qk_pass(m, half, ti, c, start, stop, pool):
                b = 3 * (m % 8) + m // 8
                sl = slice(half * NH, (half + 1) * NH)
                key = (m, half)
                if key not in pro:
                    pro[key] = pool.tile([P, NH], f32, tag=pool.name,
                                         name=f"psqk{m}_{half}")
                xv, wv = TERMS[ti]
                nc.tensor.matmul(
                    pro[key][:],
                    lhsT=wv[:, b, 2 * c:2 * c + 2, :],
                    rhs=xv[:, 2 * c:2 * c + 2, sl],
                    start=start, stop=stop,
                    perf_mode=DR,
                )
                if stop:
                    nc.vector.tensor_copy(QK[m][:, sl], pro[key][:])

            PRO_UNITS = ((0, 0, psA), (8, 0, psA), (8, 1, pss))
            for c in range(2):
                for m, h, pool in PRO_UNITS:
                    qk_pass(m, h, 0, c, start=(c == 0), stop=False, pool=pool)
            for c in range(2):
                for m, h, pool in PRO_UNITS:
                    qk_pass(m, h, 1, c, False, False, pool)
            for c in range(2, 4):
                for ti in (0, 1):
                    for m, h, pool in PRO_UNITS:
                        qk_pass(m, h, ti, c, False, False, pool)
            for c in range(4):
                for m, h, pool in PRO_UNITS:
                    qk_pass(m, h, 2, c, False,
                            stop=(c == 3), pool=pool)
            emit_v_unit(0, 0)

            for p in range(8):
                for half in range(2):
                    if p == 7 and half == 0:
                        drain(1e9, fill_q)   # flush any unemitted QK/V fill
                        state["credit"] = 0.0
                    last = (p == 7 and half == 1)
                    if last:
                        stream(p, half, proj_q, fill_ns=2 * PROJ_COST)
                    else:
                        stream(p, half, fill_q)
            drain(1e9, proj_q)

            # epilogue: proj tiles t>=4.  All 8 PSUM banks are free now:
            # open all 8 tiles' groups and run the DR + k6 passes (no pair-7
            # dependency) first; the k7 closers then pipeline right after
            # norm(7, h1) lands.  pso-pool tiles go last so their allocation
            # (which waits on the stream-7 norm reads) is hidden.
            ep_ps = {}
            ep_tiles = [(t, j) for t in range(4, TT) for j in range(2)]
            ep_pools = [psA, psA, pss, pss, pso, pso, pso, pso]

            def ep_open(t, j, pool):
                ps = pool.tile([P, NH], f32, tag=pool.name, name=f"ep{t}_{j}")
                ep_ps[t, j] = ps
                for ti, (av, wv) in enumerate(PTERMS):
                    for c in range(3):
                        nc.tensor.matmul(
                            ps[:],
                            lhsT=av[c][:, :, t * P:(t + 1) * P],
                            rhs=wv[:, 2 * c:2 * c + 2, j * NH:(j + 1) * NH],
                            start=(ti == 0 and c == 0), stop=False,
                            perf_mode=DR,
                        )
                nc.tensor.matmul(
                    ps[:], lhsT=AF[0][:, t * P:(t + 1) * P],
                    rhs=WPFv[:, 0, j * NH:(j + 1) * NH],
                    start=False, stop=False,
                )

            def ep_close_pair(t):
                """close both j-halves of token tile t; the two ob halves are
                copied on DVE and ACT in parallel and ship as ONE dma."""
                ob = obp.tile([P, 2 * NH], f16, tag="ob", name=f"eob{t}")
                for j in range(2):
                    ps = ep_ps[t, j]
                    nc.tensor.matmul(
                        ps[:], lhsT=AF[1][:, t * P:(t + 1) * P],
                        rhs=WPFv[:, 1, j * NH:(j + 1) * NH],
                        start=False, stop=True,
                    )
                    sl = slice(j * NH, (j + 1) * NH)
                    if j == 0:
                        nc.vector.tensor_scalar_mul(ob[:, sl], ps[:],
                                                    1.0 / (WS * WS))
                    else:
                        nc.scalar.mul(ob[:, sl], ps[:], 1.0 / (WS * WS))
                    if use_bias:
                        nc.vector.tensor_add(ob[:, sl], ob[:, sl],
                                             bbc[:, j * NH:(j + 1) * NH])
                (nc.scalar if t % 2 else nc.sync).dma_start(
                    out=out[t * P:(t + 1) * P, :], in_=ob[:])

            # window: open tiles 0..3, then close per-t while opening the rest
            for (t, j), pool in zip(ep_tiles[:4], ep_pools[:4]):
                ep_open(t, j, pool)
            for i in range(4, 8):
                ep_open(*ep_tiles[i], ep_pools[i])
                if i % 2 == 1:
                    ep_close_pair(ep_tiles[i - 5][0])
            ep_close_pair(6)
            ep_close_pair(7)


# revision 4
# speedup vs baseline: 1.0084x; 1.0084x over previous
"""Trainium2 Bass kernel for a dense 16-head attention block (v2).

Per batch b (data-parallel over 8 NeuronCores, no collectives):
    qkv = x @ w_qkv; q,k,v split into 16 heads of 64
    attn = softmax((q*scale) @ k.T); out = (attn @ v) @ w_proj + b_proj

Layout: everything transposed (feature dim on partitions, tokens on the free
axis) so all matmuls contract over the partition dim.

Schedule / precision design (driven by the TimelineSim cost model, where a
matmul costs out_free_size * pe_cycle * cycles_per_row, fp8e4 DoubleRow
costing 0.5 cycles/row with 2x128 contraction per pass):

  - QKV generation runs as 3-term fp8 DoubleRow: host splits x^T and
    16*w_qkv into fp8e4m3 value+residual pairs (x8,xr / w8,wr) and the
    kernel computes x8@w8 + x8@wr + xr@w8 (the dropped xr@wr term is
    O(eps^2)).  The 16x weight scale keeps w_qkv out of the fp8 subnormal
    range; it is compensated for free: the softmax "ones" columns are 16.0
    (denominator absorbs the V scale) and the exp scale absorbs 1/256 for
    the Q.K logits.
  - S^T / attn@V matmuls are fp16; proj contracts chunks 0..5 as 3-term
    fp8 DoubleRow (the normalization writes fp8 value+residual via Pool
    copy/sub) and chunks 6,7 (the last two head pairs) as fp16 so the
    final norms skip the quantize chain (rel err ~2.9e-3 overall).
  - Attention runs in (head-pair, q-half) streams: S^T -> exp (ScalarE,
    [128,512] tiles) -> attn@V accumulate with a 1-iteration software
    pipeline; the softmax denominator rides free in PSUM rows 64..127 via
    the ones columns.  Normalization is DVE reciprocal + multiply.
  - The ACT-bound inner loop leaves ~370ns/iter of PE idle; a deadline-
    ordered fill queue of deferred QK/V generation units (and, for the
    last pair, proj tiles t=0..3 which only need q-half-0 norms) keeps the
    PE busy.  A dummy-matmul warmup chain covers the DMA prologue and the
    PE p-state ramp.  Outputs ship as fp16 (host casts back to fp32) to
    halve the serialized output DMA in the epilogue tail.

PSUM budget (8 banks): pss 2 x [128,512] (S tiles), psA 2 x [128,512]
(QK/V fill + proj), pso 4 x [128,512] (AV accumulators, two streams).
"""

import numpy as np
import ml_dtypes

P = 128
N = 1024          # tokens per core (= seq len)
D = 1024          # model dim
H = 16            # heads
DH = D // H       # 64
NCORES = 8
KD = D // P       # 8 contraction chunks
TT = N // P       # 8 token chunks
NH = 512          # matmul free-dim chunk
WS = 16.0         # host-side w_qkv scale (fp8 subnormal avoidance)
SCALE = (DH ** -0.5) / (WS * WS)   # exp scale absorbs the 16x q and k scales

NWARM = 24        # warmup matmuls covering DMA prologue + p-state ramp
FILL_PER_ITER = 480.0   # ns of fill-queue work drained per inner iteration

_BF16 = ml_dtypes.bfloat16
_F8 = ml_dtypes.float8_e4m3
_F16 = np.float16

_runner_cache = {}
DEBUG_DUMP = False


def _build_nc(use_mask: bool, use_bias: bool):
    import concourse.bass as bass
    import concourse.mybir as mybir
    import concourse.tile as tile
    from concourse import bacc

    f16 = mybir.dt.float16
    f8 = mybir.dt.float8e4
    f32 = mybir.dt.float32
    Exp = mybir.ActivationFunctionType.Exp
    DR = mybir.MatmulPerfMode.DoubleRow

    nc = bacc.Bacc("TRN2", target_bir_lowering=False, debug=False)

    # DRAM inputs.  x8/xr: x^T chunk-major [128, kc*1024+tok].
    # w8/wr: 16*w_qkv as [128, b*1024 + kc*128 + c] with col-blocks of 128 in
    # PAIR-MAJOR order b = 3*p + j for original block m = p + 8*j, so each
    # head pair's Q/K/V blocks are one contiguous 3KB DMA.
    x8d = nc.dram_tensor("x8", [P, KD * N], f8, kind="ExternalInput")
    xrd = nc.dram_tensor("xr", [P, KD * N], f8, kind="ExternalInput")
    w8d = nc.dram_tensor("w8", [P, 24 * KD * P], f8, kind="ExternalInput")
    wrd = nc.dram_tensor("wr", [P, 24 * KD * P], f8, kind="ExternalInput")
    wp8d = nc.dram_tensor("wp8", [P, KD * D], f8, kind="ExternalInput")
    wprd = nc.dram_tensor("wpr", [P, KD * D], f8, kind="ExternalInput")
    wpfd = nc.dram_tensor("wpf", [P, 2 * D], f16, kind="ExternalInput")
    if use_mask:
        # masked-softmax: exp(S)*m + (1-m) == exp(where(m, S, 0)); a fully
        # masked query row softmaxes to uniform, matching the reference.
        mask_bc = nc.dram_tensor("mask_bc", [P, N], f16, kind="ExternalInput")
        imask_bc = nc.dram_tensor("imask_bc", [P, N], f16, kind="ExternalInput")
    if use_bias:
        b_bc = nc.dram_tensor("b_bc", [P, D], f32, kind="ExternalInput")
    out = nc.dram_tensor("out", [N, D], f16, kind="ExternalOutput")
    if DEBUG_DUMP:
        qk_dbg = nc.dram_tensor("qk_dbg", [P, N], f16, kind="ExternalOutput")
        kk_dbg = nc.dram_tensor("kk_dbg", [P, N], f16, kind="ExternalOutput")
        v_dbg = nc.dram_tensor("v_dbg", [P, 2 * P], f16, kind="ExternalOutput")
        aot_dbg = nc.dram_tensor("aot_dbg", [P, N], f8, kind="ExternalOutput")

    with tile.TileContext(nc) as tc:
        with (
            tc.tile_pool(name="persist", bufs=1) as pp,
            tc.tile_pool(name="pt", bufs=6) as ptp,
            tc.tile_pool(name="nrm", bufs=4) as nrm,
            tc.tile_pool(name="ob", bufs=4) as obp,
            tc.tile_pool(name="psA", bufs=2, space="PSUM") as psA,
            tc.tile_pool(name="pss", bufs=2, space="PSUM") as pss,
            tc.tile_pool(name="pso", bufs=4, space="PSUM") as pso,
        ):
            X8 = pp.tile([P, KD * N], f8, name="x8t")
            XR = pp.tile([P, KD * N], f8, name="xrt")
            W8 = pp.tile([P, 24 * KD * P], f8, name="w8t")
            WR = pp.tile([P, 24 * KD * P], f8, name="wrt")
            WP8 = pp.tile([P, KD * D], f8, name="wp8t")
            WPR = pp.tile([P, KD * D], f8, name="wprt")
            WPF = pp.tile([P, 2 * D], f16, name="wpft")
            QK = [pp.tile([P, N], f16, name=f"qk{m}") for m in range(16)]
            # V per (token tile, head pair) so just-in-time generation writes
            # don't alias the AV reads of other pairs' blocks
            V = [[pp.tile([P, 2 * P], f16, name=f"v{t}_{p}") for p in range(8)]
                 for t in range(TT)]
            # 16x-scaled attention output: chunk-pairs 0..2 (pairs 0..5) as
            # fp8 value+residual for the DR proj passes; pairs 6,7 stay fp16
            # so the last norms skip the quantize chain entirely
            A8 = [pp.tile([P, 2 * N], f8, name=f"a8t{c}") for c in range(3)]
            AR = [pp.tile([P, 2 * N], f8, name=f"art{c}") for c in range(3)]
            AF = [pp.tile([P, N], f16, name=f"aft{k}") for k in (6, 7)]
            wa = pp.tile([P, P], f16, name="warm_a")
            nc.vector.memset(wa[:], 0.0)

            # DMA order = first-use order, with few large transfers (each
            # dma_start costs ~565ns of serial SP issue time): x8 halves +
            # pair-0 weights first, then xr, then per-pair weight blocks.
            def dma_wpair(tile_, dram, p, eng=None):
                sl = slice(3 * p * KD * P, 3 * (p + 1) * KD * P)
                (eng or nc.sync).dma_start(out=tile_[:, sl], in_=dram[:, sl])

            # x8/xr split by TOKEN half (strided over the 8 chunks): the
            # prologue units only touch q-half-0, so term-3 data lands early
            def dma_xpart(tile_, dram, h, c0, c1):
                sl_t = slice(h * NH, h * NH + NH)
                tv = tile_.rearrange("p (kc t) -> p kc t", t=N)
                dv = dram.rearrange("p (kc t) -> p kc t", t=N)
                nc.sync.dma_start(out=tv[:, c0:c1, sl_t], in_=dv[:, c0:c1, sl_t])

            dma_xpart(X8, x8d, 0, 0, 4)
            dma_wpair(W8, w8d, 0)
            dma_xpart(X8, x8d, 0, 4, 8)
            dma_wpair(WR, wrd, 0)
            dma_xpart(XR, xrd, 0, 0, 8)
            dma_xpart(X8, x8d, 1, 0, 8)
            dma_xpart(XR, xrd, 1, 0, 8)
            for p in range(1, 8):
                dma_wpair(W8, w8d, p)
                dma_wpair(WR, wrd, p)
            nc.sync.dma_start(out=WP8[:], in_=wp8d[:])
            nc.sync.dma_start(out=WPR[:], in_=wprd[:])
            nc.sync.dma_start(out=WPF[:], in_=wpfd[:])
            if use_mask:
                mbc = pp.tile([P, N], f16, name="mbc")
                nc.sync.dma_start(out=mbc[:], in_=mask_bc[:])
                imbc = pp.tile([P, N], f16, name="imbc")
                nc.sync.dma_start(out=imbc[:], in_=imask_bc[:])
            if use_bias:
                bbc = pp.tile([P, D], f32, name="bbc")
                nc.sync.dma_start(out=bbc[:], in_=b_bc[:])

            # ones columns of the V tiles, on the otherwise-idle Pool
            # (1.0, not WS: leaves AOT scaled by 16x = WS for the fp8 split)
            for t in range(TT):
                for p in range(8):
                    ones_view = V[t][p].rearrange("p (h c) -> p h c", c=P)[:, :, DH:]
                    nc.gpsimd.memset(ones_view, 1.0)

            # p-state warmup: dummy matmul chain on the PE during DMA wait
            wps = pso.tile([P, P], f32, tag="pso", name="warm_ps")
            for _ in range(NWARM):
                nc.tensor.matmul(wps[:], lhsT=wa[:], rhs=wa[:],
                                 start=True, stop=True)

            X8v = X8.rearrange("p (kc t) -> p kc t", t=N)
            XRv = XR.rearrange("p (kc t) -> p kc t", t=N)
            W8v = W8.rearrange("p (m kc c) -> p m kc c", kc=KD, c=P)
            WRv = WR.rearrange("p (m kc c) -> p m kc c", kc=KD, c=P)
            WP8v = WP8.rearrange("p (k d) -> p k d", d=D)
            WPRv = WPR.rearrange("p (k d) -> p k d", d=D)
            A8v = [a.rearrange("p (i t) -> p i t", t=N) for a in A8]
            ARv = [a.rearrange("p (i t) -> p i t", t=N) for a in AR]
            WPFv = WPF.rearrange("p (k d) -> p k d", d=D)

            TERMS = ((X8v, W8v), (X8v, WRv), (XRv, W8v))

            def emit_qk_unit(m, half):
                """QK tile m (of 16), token half: 12 DoubleRow matmuls."""
                b = 3 * (m % 8) + m // 8   # pair-major weight block index
                sl = slice(half * NH, (half + 1) * NH)
                ps = psA.tile([P, NH], f32, tag="psA", name=f"psqk{m}_{half}")
                last = len(TERMS) - 1
                for ti, (xv, wv) in enumerate(TERMS):
                    for c in range(KD // 2):
                        nc.tensor.matmul(
                            ps[:],
                            lhsT=wv[:, b, 2 * c:2 * c + 2, :],
                            rhs=xv[:, 2 * c:2 * c + 2, sl],
                            start=(ti == 0 and c == 0),
                            stop=(ti == last and c == KD // 2 - 1),
                            perf_mode=DR,
                        )
                nc.vector.tensor_copy(QK[m][:, sl], ps[:])

            def emit_v_unit(t, p):
                """V cols for head pair p, token tile t: 12 tiny DR matmuls."""
                ps = psA.tile([P, P], f32, tag="psA", name=f"psv{t}_{p}")
                last = len(TERMS) - 1
                for ti, (xv, wv) in enumerate(TERMS):
                    for c in range(KD // 2):
                        nc.tensor.matmul(
                            ps[:],
                            lhsT=xv[:, 2 * c:2 * c + 2, t * P:(t + 1) * P],
                            rhs=wv[:, 3 * p + 2, 2 * c:2 * c + 2, :],
                            start=(ti == 0 and c == 0),
                            stop=(ti == last and c == KD // 2 - 1),
                            perf_mode=DR,
                        )
                dest = V[t][p].rearrange("p (h c) -> p h c", c=P)[:, :, :DH]
                nc.vector.tensor_copy(dest, ps.rearrange("p (i c) -> p i c", c=DH))

            PTERMS = ((A8v, WP8v), (A8v, WPRv), (ARv, WP8v))

            def emit_proj(t, j):
                """proj output tile: tokens t*128.., dims j*512..: chunks 0..5
                as 9 DR mm on the 16x-scaled fp8 value+residual pairs, chunks
                6,7 as fp16 mm on the 16x-scaled AOT, then a 1/256 scale."""
                ps = psA.tile([P, NH], f32, tag="psA", name=f"ps3_{t}_{j}")
                for ti, (av, wv) in enumerate(PTERMS):
                    for c in range(3):
                        nc.tensor.matmul(
                            ps[:],
                            lhsT=av[c][:, :, t * P:(t + 1) * P],
                            rhs=wv[:, 2 * c:2 * c + 2, j * NH:(j + 1) * NH],
                            start=(ti == 0 and c == 0),
                            stop=False,
                            perf_mode=DR,
                        )
                for k in (6, 7):
                    nc.tensor.matmul(
                        ps[:],
                        lhsT=AF[k - 6][:, t * P:(t + 1) * P],
                        rhs=WPFv[:, k - 6, j * NH:(j + 1) * NH],
                        start=False, stop=(k == 7),
                    )
                ob = obp.tile([P, NH], f16, tag="ob", name=f"ob{t}_{j}")
                nc.vector.tensor_scalar_mul(ob[:], ps[:], 1.0 / (WS * WS))
                if use_bias:
                    nc.vector.tensor_add(ob[:], ob[:], bbc[:, j * NH:(j + 1) * NH])
                nc.sync.dma_start(out=out[t * P:(t + 1) * P, j * NH:(j + 1) * NH],
                                  in_=ob[:])

            # ---- fill queue: deferred work units in deadline order ----
            QK_COST, V_COST, PROJ_COST = 1280.0, 320.0, 1350.0
            fill_q = []
            for t in range(1, TT):
                fill_q.append((V_COST, (emit_v_unit, t, 0)))
            fill_q.append((QK_COST, (emit_qk_unit, 0, 1)))
            for p in range(1, 8):
                fill_q.append((QK_COST, (emit_qk_unit, p, 0)))
                fill_q.append((QK_COST, (emit_qk_unit, 8 + p, 0)))
                fill_q.append((QK_COST, (emit_qk_unit, 8 + p, 1)))
                for t in range(TT):
                    fill_q.append((V_COST, (emit_v_unit, t, p)))
                fill_q.append((QK_COST, (emit_qk_unit, p, 1)))
            fill_q.reverse()   # pop from the end
            # proj tiles t<4 only need q-half-0 norms: they fill the last
            # pair's otherwise-dry streams (gated to after norm(7, h0))
            proj_q = [(PROJ_COST, (emit_proj, t, j))
                      for t in range(3, -1, -1) for j in (1, 0)]

            state = {"credit": 0.0}

            def drain(ns, q=fill_q):
                state["credit"] += ns
                while q and q[-1][0] <= state["credit"]:
                    cost, (fn, *args) = q.pop()
                    fn(*args)
                    state["credit"] -= cost

            def stream(p, half, fillq, fill_ns=FILL_PER_ITER):
                """Attention for head pair p, query half: S->exp->AV over kt
                (1-iter software pipeline), then normalize into AOT[p]."""
                sl = slice(half * NH, (half + 1) * NH)
                qtile, ktile = QK[p], QK[8 + p]
                psos = [pso.tile([P, NH], f32, tag="pso", name=f"pso{p}_{half}_{i}")
                        for i in range(2)]
                pts = {}
                for kt in range(TT + 1):
                    if kt < TT:
                        for i in range(2):
                            pr = i * DH
                            ps_s = pss.tile([P, NH], f32, tag="pss",
                                            name=f"pss{p}_{half}_{kt}_{i}")
                            nc.tensor.matmul(
                                ps_s[:],
                                lhsT=ktile[pr:pr + DH, kt * P:(kt + 1) * P],
                                rhs=qtile[pr:pr + DH, sl],
                                start=True, stop=True,
                            )
                            pt = ptp.tile([P, NH], f16, tag="pt",
                                          name=f"pt{p}_{half}_{kt}_{i}")
                            nc.scalar.activation(pt[:], ps_s[:], Exp, scale=SCALE)
                            if use_mask:
                                nc.vector.tensor_mul(pt[:], pt[:], mbc[:, sl])
                                nc.vector.tensor_add(pt[:], pt[:], imbc[:, sl])
                            pts[kt, i] = pt
                    if kt > 0:
                        for i in range(2):
                            nc.tensor.matmul(
                                psos[i][:],
                                lhsT=V[kt - 1][p][:, i * P:(i + 1) * P],
                                rhs=pts.pop((kt - 1, i))[:],
                                start=(kt - 1 == 0), stop=(kt - 1 == TT - 1),
                            )
                    drain(fill_ns, fillq)
                if p < 6:
                    # full-height staging tile: head i's rows live at base
                    # partition i*64 so the SB+SB quantize ops are aligned
                    t32 = nrm.tile([P, NH], f32, tag="t32",
                                   name=f"t32_{p}_{half}")
                for i in range(2):
                    pr = i * DH
                    rec = nrm.tile([DH, NH], f32, tag="rec",
                                   name=f"rec{p}_{half}_{i}")
                    nc.vector.reciprocal(rec[:], psos[i][DH:2 * DH, :])
                    if p >= 6:
                        nc.vector.tensor_mul(AF[p - 6][pr:pr + DH, sl],
                                             psos[i][:DH, :], rec[:])
                    else:
                        nc.vector.tensor_mul(t32[pr:pr + DH, :],
                                             psos[i][:DH, :], rec[:])
                        a8sl = A8v[p // 2][pr:pr + DH, p % 2, sl]
                        nc.gpsimd.tensor_copy(a8sl, t32[pr:pr + DH, :])
                        nc.gpsimd.tensor_sub(ARv[p // 2][pr:pr + DH, p % 2, sl],
                                             t32[pr:pr + DH, :], a8sl)

            # prologue compute: pair 0's Q,K (half 0 + K half 1) and V(t=0).
            # Four psum groups (psA x2 + idle pss x2) stay open and the fp8
            # term-passes are interleaved chunk-first, so matmuls start as
            # soon as each DMA lands instead of waiting for xr.
            pro = {}

            def qk_pass(m, half, ti, c, start, stop, pool):
                b = 3 * (m % 8) + m // 8
                sl = slice(half * NH, (half + 1) * NH)
                key = (m, half)
                if key not in pro:
                    pro[key] = pool.tile([P, NH], f32, tag=pool.name,
                                         name=f"psqk{m}_{half}")
                xv, wv = TERMS[ti]
                nc.tensor.matmul(
                    pro[key][:],
                    lhsT=wv[:, b, 2 * c:2 * c + 2, :],
                    rhs=xv[:, 2 * c:2 * c + 2, sl],
                    start=start, stop=stop,
                    perf_mode=DR,
                )
                if stop:
                    nc.vector.tensor_copy(QK[m][:, sl], pro[key][:])

            PRO_UNITS = ((0, 0, psA), (8, 0, psA), (8, 1, pss))
            for c in range(2):
                for m, h, pool in PRO_UNITS:
                    qk_pass(m, h, 0, c, start=(c == 0), stop=False, pool=pool)
            for c in range(2):
                for m, h, pool in PRO_UNITS:
                    qk_pass(m, h, 1, c, False, False, pool)
            for c in range(2, 4):
                for ti in (0, 1):
                    for m, h, pool in PRO_UNITS:
                        qk_pass(m, h, ti, c, False, False, pool)
            for c in range(4):
                for m, h, pool in PRO_UNITS:
                    qk_pass(m, h, 2, c, False,
                            stop=(c == 3), pool=pool)
            emit_v_unit(0, 0)

            for p in range(8):
                for half in range(2):
                    if p == 7 and half == 0:
                        drain(1e9, fill_q)   # flush any unemitted QK/V fill
                        state["credit"] = 0.0
                    last = (p == 7 and half == 1)
                    if last:
                        stream(p, half, proj_q, fill_ns=2 * PROJ_COST)
                    else:
                        stream(p, half, fill_q)
            drain(1e9, proj_q)

            # epilogue: proj tiles t>=4.  All 8 PSUM banks are free now:
            # open all 8 tiles' groups and run the DR + k6 passes (no pair-7
            # dependency) first; the k7 closers then pipeline right after
            # norm(7, h1) lands.  pso-pool tiles go last so their allocation
            # (which waits on the stream-7 norm reads) is hidden.
            ep_ps = {}
            ep_tiles = [(t, j) for t in range(4, TT) for j in range(2)]
            ep_pools = [psA, psA, pss, pss, pso, pso, pso, pso]

            def ep_open(t, j, pool):
                ps = pool.tile([P, NH], f32, tag=pool.name, name=f"ep{t}_{j}")
                ep_ps[t, j] = ps
                for ti, (av, wv) in enumerate(PTERMS):
                    for c in range(3):
                        nc.tensor.matmul(
                            ps[:],
                            lhsT=av[c][:, :, t * P:(t + 1) * P],
                            rhs=wv[:, 2 * c:2 * c + 2, j * NH:(j + 1) * NH],
                            start=(ti == 0 and c == 0), stop=False,
                            perf_mode=DR,
                        )
                nc.tensor.matmul(
                    ps[:], lhsT=AF[0][:, t * P:(t + 1) * P],
                    rhs=WPFv[:, 0, j * NH:(j + 1) * NH],
                    start=False, stop=False,
                )

            def ep_close_pair(t):
                """close both j-halves of token tile t; the two ob halves are
                copied on DVE and ACT in parallel and ship as ONE dma."""
                ob = obp.tile([P, 2 * NH], f16, tag="ob", name=f"eob{t}")
                for j in range(2):
                    ps = ep_ps[t, j]
                    nc.tensor.matmul(
                        ps[:], lhsT=AF[1][:, t * P:(t + 1) * P],
                        rhs=WPFv[:, 1, j * NH:(j + 1) * NH],
                        start=False, stop=True,
                    )
                    sl = slice(j * NH, (j + 1) * NH)
                    if j == 0:
                        nc.vector.tensor_scalar_mul(ob[:, sl], ps[:],
                                                    1.0 / (WS * WS))
                    else:
                        nc.scalar.mul(ob[:, sl], ps[:], 1.0 / (WS * WS))
                    if use_bias:
                        nc.vector.tensor_add(ob[:, sl], ob[:, sl],
                                             bbc[:, j * NH:(j + 1) * NH])
                nc.sync.dma_start(out=out[t * P:(t + 1) * P, :], in_=ob[:])

            # window: open tiles 0..3, then close per-t while opening the rest
            for (t, j), pool in zip(ep_tiles[:4], ep_pools[:4]):
                ep_open(t, j, pool)
            for i in range(4, 8):
                ep_open(*ep_tiles[i], ep_pools[i])
                if i % 2 == 1:
                    ep_close_pair(ep_tiles[i - 5][0])
            ep_close_pair(6)
            ep_close_pair(7)


# revision 6
# speedup vs baseline: 1.0135x; 1.0050x over previous
"""Trainium2 Bass kernel for a dense 16-head attention block (v2).

Per batch b (data-parallel over 8 NeuronCores, no collectives):
    qkv = x @ w_qkv; q,k,v split into 16 heads of 64
    attn = softmax((q*scale) @ k.T); out = (attn @ v) @ w_proj + b_proj

Layout: everything transposed (feature dim on partitions, tokens on the free
axis) so all matmuls contract over the partition dim.

Schedule / precision design (driven by the TimelineSim cost model, where a
matmul costs out_free_size * pe_cycle * cycles_per_row, fp8e4 DoubleRow
costing 0.5 cycles/row with 2x128 contraction per pass):

  - QKV generation runs as 3-term fp8 DoubleRow: host splits x^T and
    16*w_qkv into fp8e4m3 value+residual pairs (x8,xr / w8,wr) and the
    kernel computes x8@w8 + x8@wr + xr@w8 (the dropped xr@wr term is
    O(eps^2)).  The 16x weight scale keeps w_qkv out of the fp8 subnormal
    range; it is compensated for free: the softmax "ones" columns are 16.0
    (denominator absorbs the V scale) and the exp scale absorbs 1/256 for
    the Q.K logits.
  - S^T / attn@V matmuls are fp16; proj contracts chunks 0..5 as 3-term
    fp8 DoubleRow (the normalization writes fp8 value+residual via Pool
    copy/sub) and chunks 6,7 (the last two head pairs) as fp16 so the
    final norms skip the quantize chain (rel err ~2.9e-3 overall).
  - Attention runs in (head-pair, q-half) streams: S^T -> exp (ScalarE,
    [128,512] tiles) -> attn@V accumulate with a 1-iteration software
    pipeline; the softmax denominator rides free in PSUM rows 64..127 via
    the ones columns.  Normalization is DVE reciprocal + multiply.
  - The ACT-bound inner loop leaves ~370ns/iter of PE idle; a deadline-
    ordered fill queue of deferred QK/V generation units (and, for the
    last pair, proj tiles t=0..3 which only need q-half-0 norms) keeps the
    PE busy.  A dummy-matmul warmup chain covers the DMA prologue and the
    PE p-state ramp.  Outputs ship as fp16 (host casts back to fp32) to
    halve the serialized output DMA in the epilogue tail.

PSUM budget (8 banks): pss 2 x [128,512] (S tiles), psA 2 x [128,512]
(QK/V fill + proj), pso 4 x [128,512] (AV accumulators, two streams).
"""

import numpy as np
import ml_dtypes

P = 128
N = 1024          # tokens per core (= seq len)
D = 1024          # model dim
H = 16            # heads
DH = D // H       # 64
NCORES = 8
KD = D // P       # 8 contraction chunks
TT = N // P       # 8 token chunks
NH = 512          # matmul free-dim chunk
WS = 16.0         # host-side w_qkv scale (fp8 subnormal avoidance)
SCALE = (DH ** -0.5) / (WS * WS)   # exp scale absorbs the 16x q and k scales

NWARM = 24        # warmup matmuls covering DMA prologue + p-state ramp
FILL_PER_ITER = 480.0   # ns of fill-queue work drained per inner iteration

_BF16 = ml_dtypes.bfloat16
_F8 = ml_dtypes.float8_e4m3
_F16 = np.float16

_runner_cache = {}
DEBUG_DUMP = False


def _build_nc(use_mask: bool, use_bias: bool):
    import concourse.bass as bass
    import concourse.mybir as mybir
    import concourse.tile as tile
    from concourse import bacc

    f16 = mybir.dt.float16
    f8 = mybir.dt.float8e4
    f32 = mybir.dt.float32
    Exp = mybir.ActivationFunctionType.Exp
    DR = mybir.MatmulPerfMode.DoubleRow

    nc = bacc.Bacc("TRN2", target_bir_lowering=False, debug=False)

    # DRAM inputs.  x8/xr: x^T chunk-major [128, kc*1024+tok].
    # w8/wr: 16*w_qkv as [128, b*1024 + kc*128 + c] with col-blocks of 128 in
    # PAIR-MAJOR order b = 3*p + j for original block m = p + 8*j, so each
    # head pair's Q/K/V blocks are one contiguous 3KB DMA.
    x8d = nc.dram_tensor("x8", [P, KD * N], f8, kind="ExternalInput")
    xrd = nc.dram_tensor("xr", [P, KD * N], f8, kind="ExternalInput")
    w8d = nc.dram_tensor("w8", [P, 24 * KD * P], f8, kind="ExternalInput")
    wrd = nc.dram_tensor("wr", [P, 24 * KD * P], f8, kind="ExternalInput")
    wp8d = nc.dram_tensor("wp8", [P, KD * D], f8, kind="ExternalInput")
    wprd = nc.dram_tensor("wpr", [P, KD * D], f8, kind="ExternalInput")
    wpfd = nc.dram_tensor("wpf", [P, 2 * D], f16, kind="ExternalInput")
    if use_mask:
        # masked-softmax: exp(S)*m + (1-m) == exp(where(m, S, 0)); a fully
        # masked query row softmaxes to uniform, matching the reference.
        mask_bc = nc.dram_tensor("mask_bc", [P, N], f16, kind="ExternalInput")
        imask_bc = nc.dram_tensor("imask_bc", [P, N], f16, kind="ExternalInput")
    if use_bias:
        b_bc = nc.dram_tensor("b_bc", [P, D], f32, kind="ExternalInput")
    out = nc.dram_tensor("out", [N, D], f16, kind="ExternalOutput")
    if DEBUG_DUMP:
        qk_dbg = nc.dram_tensor("qk_dbg", [P, N], f16, kind="ExternalOutput")
        kk_dbg = nc.dram_tensor("kk_dbg", [P, N], f16, kind="ExternalOutput")
        v_dbg = nc.dram_tensor("v_dbg", [P, 2 * P], f16, kind="ExternalOutput")
        aot_dbg = nc.dram_tensor("aot_dbg", [P, N], f8, kind="ExternalOutput")

    with tile.TileContext(nc) as tc:
        with (
            tc.tile_pool(name="persist", bufs=1) as pp,
            tc.tile_pool(name="pt", bufs=6) as ptp,
            tc.tile_pool(name="nrm", bufs=4) as nrm,
            tc.tile_pool(name="ob", bufs=4) as obp,
            tc.tile_pool(name="psA", bufs=2, space="PSUM") as psA,
            tc.tile_pool(name="pss", bufs=2, space="PSUM") as pss,
            tc.tile_pool(name="pso", bufs=4, space="PSUM") as pso,
        ):
            X8 = pp.tile([P, KD * N], f8, name="x8t")
            XR = pp.tile([P, KD * N], f8, name="xrt")
            W8 = pp.tile([P, 24 * KD * P], f8, name="w8t")
            WR = pp.tile([P, 24 * KD * P], f8, name="wrt")
            WP8 = pp.tile([P, KD * D], f8, name="wp8t")
            WPR = pp.tile([P, KD * D], f8, name="wprt")
            WPF = pp.tile([P, 2 * D], f16, name="wpft")
            QK = [pp.tile([P, N], f16, name=f"qk{m}") for m in range(16)]
            # V per (token tile, head pair) so just-in-time generation writes
            # don't alias the AV reads of other pairs' blocks
            V = [[pp.tile([P, 2 * P], f16, name=f"v{t}_{p}") for p in range(8)]
                 for t in range(TT)]
            # 16x-scaled attention output: chunk-pairs 0..2 (pairs 0..5) as
            # fp8 value+residual for the DR proj passes; pairs 6,7 stay fp16
            # so the last norms skip the quantize chain entirely
            A8 = [pp.tile([P, 2 * N], f8, name=f"a8t{c}") for c in range(3)]
            AR = [pp.tile([P, 2 * N], f8, name=f"art{c}") for c in range(3)]
            AF = [pp.tile([P, N], f16, name=f"aft{k}") for k in (6, 7)]
            wa = pp.tile([P, P], f16, name="warm_a")
            nc.vector.memset(wa[:], 0.0)

            # DMA order = first-use order, with few large transfers (each
            # dma_start costs ~565ns of serial SP issue time): x8 halves +
            # pair-0 weights first, then xr, then per-pair weight blocks.
            def dma_wpair(tile_, dram, p, eng=None):
                sl = slice(3 * p * KD * P, 3 * (p + 1) * KD * P)
                (eng or nc.sync).dma_start(out=tile_[:, sl], in_=dram[:, sl])

            # x8/xr split by TOKEN half (strided over the 8 chunks): the
            # prologue units only touch q-half-0, so term-3 data lands early
            def dma_xpart(tile_, dram, h, c0, c1):
                sl_t = slice(h * NH, h * NH + NH)
                tv = tile_.rearrange("p (kc t) -> p kc t", t=N)
                dv = dram.rearrange("p (kc t) -> p kc t", t=N)
                nc.sync.dma_start(out=tv[:, c0:c1, sl_t], in_=dv[:, c0:c1, sl_t])

            def dma_wblk(tile_, dram, b0, b1):
                sl = slice(b0 * KD * P, b1 * KD * P)
                nc.sync.dma_start(out=tile_[:, sl], in_=dram[:, sl])

            # pair-0 weights split QK-blocks vs V-block (V needed ~6us later),
            # first xr half split so term-3 passes start sooner
            dma_xpart(X8, x8d, 0, 0, 4)
            dma_wblk(W8, w8d, 0, 2)
            dma_wblk(WR, wrd, 0, 2)
            dma_xpart(X8, x8d, 0, 4, 8)
            dma_xpart(XR, xrd, 0, 0, 4)
            dma_xpart(XR, xrd, 0, 4, 8)
            dma_wblk(W8, w8d, 2, 3)
            dma_wblk(WR, wrd, 2, 3)
            dma_xpart(X8, x8d, 1, 0, 8)
            dma_xpart(XR, xrd, 1, 0, 8)
            for p in range(1, 8):
                dma_wpair(W8, w8d, p)
                dma_wpair(WR, wrd, p)
            nc.sync.dma_start(out=WP8[:], in_=wp8d[:])
            nc.sync.dma_start(out=WPR[:], in_=wprd[:])
            nc.sync.dma_start(out=WPF[:], in_=wpfd[:])
            if use_mask:
                mbc = pp.tile([P, N], f16, name="mbc")
                nc.sync.dma_start(out=mbc[:], in_=mask_bc[:])
                imbc = pp.tile([P, N], f16, name="imbc")
                nc.sync.dma_start(out=imbc[:], in_=imask_bc[:])
            if use_bias:
                bbc = pp.tile([P, D], f32, name="bbc")
                nc.sync.dma_start(out=bbc[:], in_=b_bc[:])

            # ones columns of the V tiles, on the otherwise-idle Pool
            # (1.0, not WS: leaves AOT scaled by 16x = WS for the fp8 split)
            for t in range(TT):
                for p in range(8):
                    ones_view = V[t][p].rearrange("p (h c) -> p h c", c=P)[:, :, DH:]
                    nc.gpsimd.memset(ones_view, 1.0)

            # p-state warmup: dummy matmul chain on the PE during DMA wait
            wps = pso.tile([P, P], f32, tag="pso", name="warm_ps")
            for _ in range(NWARM):
                nc.tensor.matmul(wps[:], lhsT=wa[:], rhs=wa[:],
                                 start=True, stop=True)

            X8v = X8.rearrange("p (kc t) -> p kc t", t=N)
            XRv = XR.rearrange("p (kc t) -> p kc t", t=N)
            W8v = W8.rearrange("p (m kc c) -> p m kc c", kc=KD, c=P)
            WRv = WR.rearrange("p (m kc c) -> p m kc c", kc=KD, c=P)
            WP8v = WP8.rearrange("p (k d) -> p k d", d=D)
            WPRv = WPR.rearrange("p (k d) -> p k d", d=D)
            A8v = [a.rearrange("p (i t) -> p i t", t=N) for a in A8]
            ARv = [a.rearrange("p (i t) -> p i t", t=N) for a in AR]
            WPFv = WPF.rearrange("p (k d) -> p k d", d=D)

            TERMS = ((X8v, W8v), (X8v, WRv), (XRv, W8v))

            def emit_qk_unit(m, half):
                """QK tile m (of 16), token half: 12 DoubleRow matmuls."""
                b = 3 * (m % 8) + m // 8   # pair-major weight block index
                sl = slice(half * NH, (half + 1) * NH)
                ps = psA.tile([P, NH], f32, tag="psA", name=f"psqk{m}_{half}")
                last = len(TERMS) - 1
                for ti, (xv, wv) in enumerate(TERMS):
                    for c in range(KD // 2):
                        nc.tensor.matmul(
                            ps[:],
                            lhsT=wv[:, b, 2 * c:2 * c + 2, :],
                            rhs=xv[:, 2 * c:2 * c + 2, sl],
                            start=(ti == 0 and c == 0),
                            stop=(ti == last and c == KD // 2 - 1),
                            perf_mode=DR,
                        )
                nc.vector.tensor_copy(QK[m][:, sl], ps[:])

            def emit_v_unit(t, p):
                """V cols for head pair p, token tile t: 12 tiny DR matmuls."""
                ps = psA.tile([P, P], f32, tag="psA", name=f"psv{t}_{p}")
                last = len(TERMS) - 1
                for ti, (xv, wv) in enumerate(TERMS):
                    for c in range(KD // 2):
                        nc.tensor.matmul(
                            ps[:],
                            lhsT=xv[:, 2 * c:2 * c + 2, t * P:(t + 1) * P],
                            rhs=wv[:, 3 * p + 2, 2 * c:2 * c + 2, :],
                            start=(ti == 0 and c == 0),
                            stop=(ti == last and c == KD // 2 - 1),
                            perf_mode=DR,
                        )
                dest = V[t][p].rearrange("p (h c) -> p h c", c=P)[:, :, :DH]
                nc.vector.tensor_copy(dest, ps.rearrange("p (i c) -> p i c", c=DH))

            PTERMS = ((A8v, WP8v), (A8v, WPRv), (ARv, WP8v))

            def emit_proj(t, j):
                """proj output tile: tokens t*128.., dims j*512..: chunks 0..5
                as 9 DR mm on the 16x-scaled fp8 value+residual pairs, chunks
                6,7 as fp16 mm on the 16x-scaled AOT, then a 1/256 scale."""
                ps = psA.tile([P, NH], f32, tag="psA", name=f"ps3_{t}_{j}")
                for ti, (av, wv) in enumerate(PTERMS):
                    for c in range(3):
                        nc.tensor.matmul(
                            ps[:],
                            lhsT=av[c][:, :, t * P:(t + 1) * P],
                            rhs=wv[:, 2 * c:2 * c + 2, j * NH:(j + 1) * NH],
                            start=(ti == 0 and c == 0),
                            stop=False,
                            perf_mode=DR,
                        )
                for k in (6, 7):
                    nc.tensor.matmul(
                        ps[:],
                        lhsT=AF[k - 6][:, t * P:(t + 1) * P],
                        rhs=WPFv[:, k - 6, j * NH:(j + 1) * NH],
                        start=False, stop=(k == 7),
                    )
                ob = obp.tile([P, NH], f16, tag="ob", name=f"ob{t}_{j}")
                nc.vector.tensor_scalar_mul(ob[:], ps[:], 1.0 / (WS * WS))
                if use_bias:
                    nc.vector.tensor_add(ob[:], ob[:], bbc[:, j * NH:(j + 1) * NH])
                nc.sync.dma_start(out=out[t * P:(t + 1) * P, j * NH:(j + 1) * NH],
                                  in_=ob[:])

            # ---- fill queue: deferred work units in deadline order ----
            QK_COST, V_COST, PROJ_COST = 1280.0, 320.0, 1350.0
            fill_q = []
            for t in range(1, TT):
                fill_q.append((V_COST, (emit_v_unit, t, 0)))
            fill_q.append((QK_COST, (emit_qk_unit, 0, 1)))
            for p in range(1, 8):
                fill_q.append((QK_COST, (emit_qk_unit, p, 0)))
                fill_q.append((QK_COST, (emit_qk_unit, 8 + p, 0)))
                fill_q.append((QK_COST, (emit_qk_unit, 8 + p, 1)))
                for t in range(TT):
                    fill_q.append((V_COST, (emit_v_unit, t, p)))
                fill_q.append((QK_COST, (emit_qk_unit, p, 1)))
            fill_q.reverse()   # pop from the end
            # proj tiles t<4 only need q-half-0 norms: they fill the last
            # pair's otherwise-dry streams (gated to after norm(7, h0))
            proj_q = [(PROJ_COST, (emit_proj, t, j))
                      for t in range(3, -1, -1) for j in (1, 0)]

            state = {"credit": 0.0}

            def drain(ns, q=fill_q):
                state["credit"] += ns
                while q and q[-1][0] <= state["credit"]:
                    cost, (fn, *args) = q.pop()
                    fn(*args)
                    state["credit"] -= cost

            def stream(p, half, fillq, fill_ns=FILL_PER_ITER):
                """Attention for head pair p, query half: S->exp->AV over kt
                (1-iter software pipeline), then normalize into AOT[p]."""
                sl = slice(half * NH, (half + 1) * NH)
                qtile, ktile = QK[p], QK[8 + p]
                psos = [pso.tile([P, NH], f32, tag="pso", name=f"pso{p}_{half}_{i}")
                        for i in range(2)]
                pts = {}
                for kt in range(TT + 1):
                    if kt < TT:
                        for i in range(2):
                            pr = i * DH
                            ps_s = pss.tile([P, NH], f32, tag="pss",
                                            name=f"pss{p}_{half}_{kt}_{i}")
                            nc.tensor.matmul(
                                ps_s[:],
                                lhsT=ktile[pr:pr + DH, kt * P:(kt + 1) * P],
                                rhs=qtile[pr:pr + DH, sl],
                                start=True, stop=True,
                            )
                            pt = ptp.tile([P, NH], f16, tag="pt",
                                          name=f"pt{p}_{half}_{kt}_{i}")
                            nc.scalar.activation(pt[:], ps_s[:], Exp, scale=SCALE)
                            if use_mask:
                                nc.vector.tensor_mul(pt[:], pt[:], mbc[:, sl])
                                nc.vector.tensor_add(pt[:], pt[:], imbc[:, sl])
                            pts[kt, i] = pt
                    if kt > 0:
                        for i in range(2):
                            nc.tensor.matmul(
                                psos[i][:],
                                lhsT=V[kt - 1][p][:, i * P:(i + 1) * P],
                                rhs=pts.pop((kt - 1, i))[:],
                                start=(kt - 1 == 0), stop=(kt - 1 == TT - 1),
                            )
                    drain(fill_ns, fillq)
                if p < 6:
                    # full-height staging tile: head i's rows live at base
                    # partition i*64 so the SB+SB quantize ops are aligned
                    t32 = nrm.tile([P, NH], f32, tag="t32",
                                   name=f"t32_{p}_{half}")
                for i in range(2):
                    pr = i * DH
                    rec = nrm.tile([DH, NH], f32, tag="rec",
                                   name=f"rec{p}_{half}_{i}")
                    nc.vector.reciprocal(rec[:], psos[i][DH:2 * DH, :])
                    if p >= 6:
                        nc.vector.tensor_mul(AF[p - 6][pr:pr + DH, sl],
                                             psos[i][:DH, :], rec[:])
                    else:
                        nc.vector.tensor_mul(t32[pr:pr + DH, :],
                                             psos[i][:DH, :], rec[:])
                        a8sl = A8v[p // 2][pr:pr + DH, p % 2, sl]
                        nc.gpsimd.tensor_copy(a8sl, t32[pr:pr + DH, :])
                        nc.gpsimd.tensor_sub(ARv[p // 2][pr:pr + DH, p % 2, sl],
                                             t32[pr:pr + DH, :], a8sl)

            # prologue compute: pair 0's Q,K (half 0 + K half 1) and V(t=0).
            # Four psum groups (psA x2 + idle pss x2) stay open and the fp8
            # term-passes are interleaved chunk-first, so matmuls start as
            # soon as each DMA lands instead of waiting for xr.
            pro = {}

            def qk_pass(m, half, ti, c, start, stop, pool):
                b = 3 * (m % 8) + m // 8
                sl = slice(half * NH, (half + 1) * NH)
                key = (m, half)
                if key not in pro:
                    pro[key] = pool.tile([P, NH], f32, tag=pool.name,
                                         name=f"psqk{m}_{half}")
                xv, wv = TERMS[ti]
                nc.tensor.matmul(
                    pro[key][:],
                    lhsT=wv[:, b, 2 * c:2 * c + 2, :],
                    rhs=xv[:, 2 * c:2 * c + 2, sl],
                    start=start, stop=stop,
                    perf_mode=DR,
                )
                if stop:
                    nc.vector.tensor_copy(QK[m][:, sl], pro[key][:])

            PRO_UNITS = ((0, 0, psA), (8, 0, psA), (8, 1, pss))
            for c in range(2):
                for m, h, pool in PRO_UNITS:
                    qk_pass(m, h, 0, c, start=(c == 0), stop=False, pool=pool)
            for c in range(2):
                for m, h, pool in PRO_UNITS:
                    qk_pass(m, h, 1, c, False, False, pool)
            for c in range(2, 4):
                for ti in (0, 1):
                    for m, h, pool in PRO_UNITS:
                        qk_pass(m, h, ti, c, False, False, pool)
            for c in range(4):
                for m, h, pool in PRO_UNITS:
                    qk_pass(m, h, 2, c, False,
                            stop=(c == 3), pool=pool)
            emit_v_unit(0, 0)

            for p in range(8):
                for half in range(2):
                    if p == 7 and half == 0:
                        drain(1e9, fill_q)   # flush any unemitted QK/V fill
                        state["credit"] = 0.0
                    last = (p == 7 and half == 1)
                    if last:
                        stream(p, half, proj_q, fill_ns=PROJ_COST)
                    else:
                        stream(p, half, fill_q)
            drain(1e9, proj_q)

            # epilogue: proj tiles t>=4.  All 8 PSUM banks are free now:
            # open all 8 tiles' groups and run the DR + k6 passes (no pair-7
            # dependency) first; the k7 closers then pipeline right after
            # norm(7, h1) lands.  pso-pool tiles go last so their allocation
            # (which waits on the stream-7 norm reads) is hidden.
            ep_ps = {}
            ep_tiles = [(t, j) for t in range(4, TT) for j in range(2)]
            ep_pools = [psA, psA, pss, pss, pso, pso, pso, pso]

            def ep_open(t, j, pool):
                ps = pool.tile([P, NH], f32, tag=pool.name, name=f"ep{t}_{j}")
                ep_ps[t, j] = ps
                for ti, (av, wv) in enumerate(PTERMS):
                    for c in range(3):
                        nc.tensor.matmul(
                            ps[:],
                            lhsT=av[c][:, :, t * P:(t + 1) * P],
                            rhs=wv[:, 2 * c:2 * c + 2, j * NH:(j + 1) * NH],
                            start=(ti == 0 and c == 0), stop=False,
                            perf_mode=DR,
                        )
                nc.tensor.matmul(
                    ps[:], lhsT=AF[0][:, t * P:(t + 1) * P],
                    rhs=WPFv[:, 0, j * NH:(j + 1) * NH],
                    start=False, stop=False,
                )

            def ep_close_pair(t):
                """close both j-halves of token tile t; the two ob halves are
                copied on DVE and ACT in parallel and ship as ONE dma."""
                ob = obp.tile([P, 2 * NH], f16, tag="ob", name=f"eob{t}")
                for j in range(2):
                    ps = ep_ps[t, j]
                    nc.tensor.matmul(
                        ps[:], lhsT=AF[1][:, t * P:(t + 1) * P],
                        rhs=WPFv[:, 1, j * NH:(j + 1) * NH],
                        start=False, stop=True,
                    )
                    sl = slice(j * NH, (j + 1) * NH)
                    if j == 0:
                        nc.vector.tensor_scalar_mul(ob[:, sl], ps[:],
                                                    1.0 / (WS * WS))
                    else:
                        nc.scalar.mul(ob[:, sl], ps[:], 1.0 / (WS * WS))
                    if use_bias:
                        nc.vector.tensor_add(ob[:, sl], ob[:, sl],
                                             bbc[:, j * NH:(j + 1) * NH])
                nc.sync.dma_start(out=out[t * P:(t + 1) * P, :], in_=ob[:])

            # window: open tiles 0..3, then close per-t while opening the rest
            for (t, j), pool in zip(ep_tiles[:4], ep_pools[:4]):
                ep_open(t, j, pool)
            for i in range(4, 8):
                ep_open(*ep_tiles[i], ep_pools[i])
                if i % 2 == 1:
                    ep_close_pair(ep_tiles[i - 5][0])
            ep_close_pair(6)
            ep_close_pair(7)
